# revision 22
# baseline (speedup 1.0000x reference)
"""Bahdanau-style attention scores kernel for 8 TRN2 NeuronCores.

Reference math (B=64, S=2048, E=512, D=512):
    Wh = attn_W[:D]; We = attn_W[D:]
    h_proj = hidden @ Wh                                  # [B, D]
    e_proj[b,s,:] = enc[b,s,:] @ We                       # [B, S, D]
    energy = tanh(h_proj[:,None,:] + e_proj + attn_b)     # [B, S, D]
    scores = energy @ v_w                                 # [B, S]
    out = softmax(scores, axis=1)

Sharding: data-parallel over batch, 8 batches per core.
Host precomputes c = hidden @ Wh + attn_b (tiny: 33 MFLOP), converts
enc/We to bf16 (halves HBM traffic; absmax_rel ~5e-3, gate is 2e-2),
and transposes enc to [b, e, s] so the contraction axis e lands on SBUF
partitions.

Per-core kernel (b = 0..7 local batches), d-outer:
  for d: 16 matmuls (4e x 4s) accumulate e_projT[d_chunk, s] into
         2-bank PSUM pair tiles. d>0 runs e-outer/s-inner so consecutive
         matmuls share lhsT; d=0 runs s-outer/e-inner so the first
         matmuls only depend on the first enc s-block DMA.
  energyT = tanh(psum + c_b[d]) -> bf16, one ACT call per [128, 1024]
  score matvecs v_d.T @ energyT accumulate into PSUM rows 0/32/64/96
  (tile_position col strips), deferred one d so they never head-of-line
  block the main matmuls.
  softmax tail on the 4 score rows, deferred one batch for overlap.
A few dummy warm-up matmuls at t=0 lift the PE HAM clock gate toward
2.4 GHz before the first real matmul's DMA dependencies land; the
startup DMAs are split fine-grained (enc/We half-chunks) so the first
matmul starts ~2 us earlier.
"""

import numpy as np

import concourse.bass as bass  # noqa: F401  (engine namespaces via nc)
import concourse.mybir as mybir
import concourse.tile as tile
from concourse import bacc, bass_isa
from concourse.bass_utils import run_bass_kernel_spmd

N_CORES = 8
B, S, E, D = 64, 2048, 512, 512
BL = B // N_CORES          # local batches per core
P = 128                    # partition tile
EC = E // P                # e chunks (4)
DC = D // P                # d chunks (4)
ST = 512                   # s tile (free dim per matmul; one PSUM bank f32)
SC = S // ST               # s tiles (4)

DT = mybir.dt.bfloat16     # matmul input dtype (enc, We, v, energy)

_COMPILED = None  # nc cache within the process


def _build(warmup=5, enc_bufs=3, psp_bufs=3, en_bufs=6, mv="pe"):
    nc = bacc.Bacc(
        "TRN2", target_bir_lowering=False, debug=False, num_devices=N_CORES
    )
    f32 = mybir.dt.float32

    enc_ap = nc.dram_tensor("enc_t", [BL, E, S], DT, kind="ExternalInput").ap()
    we_ap = nc.dram_tensor("we", [E, D], DT, kind="ExternalInput").ap()
    c_ap = nc.dram_tensor("c", [P, BL * DC], f32, kind="ExternalInput").ap()
    v_ap = nc.dram_tensor("v", [P, DC], DT, kind="ExternalInput").ap()
    vb_ap = nc.dram_tensor("vb", [P, DC * ST], DT, kind="ExternalInput").ap()
    out_ap = nc.dram_tensor("out", [BL, S], f32, kind="ExternalOutput").ap()

    with tile.TileContext(nc) as tc:
        with (
            tc.tile_pool(name="singles", bufs=1) as singles,
            tc.tile_pool(name="encp", bufs=enc_bufs) as encp,
            tc.tile_pool(name="enp", bufs=en_bufs) as enp,
            tc.tile_pool(name="envp", bufs=4) as envp,
            tc.tile_pool(name="accp", bufs=8) as accp,
            tc.tile_pool(name="expp", bufs=2) as expp,
            tc.tile_pool(name="smallp", bufs=2) as smallp,
            tc.tile_pool(name="outp", bufs=2) as outp,
            tc.tile_pool(name="warmp", bufs=1) as warmp,
            tc.tile_pool(name="psp", bufs=psp_bufs, space="PSUM") as psp,
            tc.tile_pool(name="scp", bufs=2, space="PSUM") as scp,
        ):
            # ---- PE warm-up: run the HAM activity window hot before the
            # first real matmul's DMA deps arrive (dummy data, never read).
            # The zeros tile doubles as the lhsT of the per-batch score-bank
            # zeroing matmul. gpsimd does the memset (its preamble retires
            # earliest; DVE's is ~1 us later).
            wsb = warmp.tile([P, ST], DT)
            nc.gpsimd.memset(wsb, 0.0)
            wps = scp.tile([P, ST], f32, name="warm_ps", tag="sc")
            for _ in range(warmup):
                nc.tensor.matmul(wps, lhsT=wsb[:, 0:P], rhs=wsb,
                                 start=True, stop=True)
            # 0/1 mask of the four score rows (partitions 0/32/64/96)
            mask = warmp.tile([P, 1], f32, name="mask", tag="mask")
            nc.vector.memset(mask, 0.0)
            for s in range(SC):
                nc.vector.memset(mask[32 * s : 32 * s + 1, :], 1.0)

            we_sb = singles.tile([P, EC, D], DT)
            c_sb = singles.tile([P, BL * DC], f32)
            v_sb = singles.tile([P, DC], DT)
            if mv == "dve":
                # v broadcast along the free axis, for tensor_tensor muls
                vb_sb = singles.tile([P, DC, ST], DT)
                ones_sb = warmp.tile([P, 1], DT, name="ones", tag="ones")
                nc.gpsimd.memset(ones_sb, 1.0)

            pend = {0: [], 1: []}  # deferred score matvecs per half
            tails = []  # deferred per-batch softmax tails

            def flush_pend(h):
                for scores_t, s, d, en_t in pend[h]:
                    if d == 0 and s == 0:
                        # zero the whole score bank (and set has_written)
                        # so the tail's exp can read all 128 rows in one go
                        nc.tensor.matmul(
                            scores_t, lhsT=wsb[:, 0:P], rhs=wsb,
                            start=True, stop=False,
                        )
                    nc.tensor.matmul(
                        scores_t[32 * s : 32 * s + 1, :],
                        lhsT=v_sb[:, d : d + 1],
                        rhs=en_t,
                        start=False,
                        stop=False,
                        tile_position=(0, 32 * s),
                    )
                    if d == DC - 1 and s == SC - 1:
                        # accumulate-zero closer: data no-op, marks every
                        # element's accumulation group closed
                        nc.tensor.matmul(
                            scores_t, lhsT=wsb[:, 0:P], rhs=wsb,
                            start=False, stop=True,
                        )
                pend[h].clear()

            def flush_pend_dve():
                # one partition-reducing ones-matvec per (batch, s-tile)
                for scores_t, s, acc_t, last_b in pend:
                    if last_b and s == 0:
                        nc.tensor.matmul(
                            scores_t, lhsT=wsb[:, 0:P], rhs=wsb,
                            start=True, stop=False,
                        )
                    nc.tensor.matmul(
                        scores_t[32 * s : 32 * s + 1, :],
                        lhsT=ones_sb[:, 0:1],
                        rhs=acc_t,
                        start=not last_b,
                        stop=not last_b,
                        tile_position=(0, 32 * s),
                    )
                    if last_b and s == SC - 1:
                        nc.tensor.matmul(
                            scores_t, lhsT=wsb[:, 0:P], rhs=wsb,
                            start=False, stop=True,
                        )
                pend.clear()

            def flush_and_tail(h, pop=False):
                flush_pend(h)
                if pop and tails:
                    tails.pop(0)()

            we_src = we_ap.rearrange("(e p) d -> p e d", p=P)

            for b in range(BL):
                enc_sb = encp.tile([P, EC, S], DT, name=f"enc_b{b}", tag="enc")
                enc_src = enc_ap[b].rearrange("(e p) s -> p e s", p=P)
                if b == 0:
                    # fine-grained startup: alternate enc-s0 / We half
                    # chunks so the first matmul's deps land after 256 KB,
                    # then c/v (small), then the remaining s-blocks
                    for e2 in range(2):
                        nc.sync.dma_start(
                            out=enc_sb[:, 2 * e2 : 2 * e2 + 2, 0:ST],
                            in_=enc_src[:, 2 * e2 : 2 * e2 + 2, 0:ST],
                        )
                        nc.sync.dma_start(
                            out=we_sb[:, 2 * e2 : 2 * e2 + 2, :],
                            in_=we_src[:, 2 * e2 : 2 * e2 + 2, :],
                        )
                    nc.sync.dma_start(out=c_sb, in_=c_ap)
                    nc.sync.dma_start(out=v_sb, in_=v_ap)
                    if mv == "dve":
                        nc.sync.dma_start(
                            out=vb_sb,
                            in_=vb_ap.rearrange("p (d t) -> p d t", d=DC),
                        )
                    for s in range(1, SC):
                        nc.sync.dma_start(
                            out=enc_sb[:, :, s * ST : (s + 1) * ST],
                            in_=enc_src[:, :, s * ST : (s + 1) * ST],
                        )
                else:
                    for h in range(2):
                        sl = slice(h * 2 * ST, (h + 1) * 2 * ST)
                        nc.sync.dma_start(
                            out=enc_sb[:, :, sl], in_=enc_src[:, :, sl]
                        )

                scores = scp.tile([P, ST], f32, name=f"scores_b{b}", tag="sc")
                if mv == "dve":
                    accs = [
                        accp.tile([P, ST], DT, name=f"acc_b{b}s{s}",
                                  tag="acc")
                        for s in range(SC)
                    ]

                for d in range(DC):
                    ps_pair = [
                        psp.tile([P, 2, ST], f32, name=f"ps_b{b}d{d}h{h}",
                                 tag="ps")
                        for h in range(2)
                    ]

                    def mm(e, s):
                        nc.tensor.matmul(
                            ps_pair[s // 2][:, s % 2, :],
                            lhsT=we_sb[:, e, d * P : (d + 1) * P],
                            rhs=enc_sb[:, e, s * ST : (s + 1) * ST],
                            start=(e == 0),
                            stop=(e == EC - 1),
                        )

                    # half-blocks: the 8 matmuls of one psum pair, then the
                    # previous d's score matvecs for that same half (their
                    # tanh inputs are long done), then the half's tanh.
                    # Keeps the PE stream free of psum-rotation and
                    # matvec-dependency stalls.
                    for h in range(2):
                        ss = (2 * h, 2 * h + 1)
                        if b == 0 and d == 0:
                            # s-outer: first matmuls only need the first
                            # enc s-block
                            order = [(e, s) for s in ss for e in range(EC)]
                        else:
                            # e-outer: consecutive matmul pairs share lhsT
                            order = [(e, s) for e in range(EC) for s in ss]
                        for e, s in order:
                            mm(e, s)
                        flush_and_tail(h, pop=(d == 1 and h == 1))
                        en = enp.tile([P, 2, ST], DT, name=f"en_b{b}d{d}h{h}",
                                      tag="en")
                        nc.scalar.activation(
                            en,
                            ps_pair[h],
                            mybir.ActivationFunctionType.Tanh,
                            bias=c_sb[:, b * DC + d : b * DC + d + 1],
                        )
                        for g in range(2):
                            s = 2 * h + g
                            pend[h].append((scores, s, d, en[:, g, :]))

                def make_tail(b=b, scores=scores):
                    def tail():
                        ssum4 = smallp.tile(
                            [P, 1], f32, name=f"ssum4_b{b}", tag="ssum4"
                        )
                        expw = expp.tile(
                            [P, ST], f32, name=f"expw_b{b}", tag="expw"
                        )
                        if True:
                            # exposed tail: all 128 score rows are defined
                            # (zero opener/closer), so exp+accumulate runs
                            # as ONE call; junk-row partials (exp(0)*512)
                            # are masked out before the partition reduce
                            nc.scalar.activation(
                                expw,
                                scores,
                                mybir.ActivationFunctionType.Exp,
                                accum_out=ssum4[:, 0:1],
                            )
                            msum = smallp.tile(
                                [P, 1], f32, name=f"msum_b{b}", tag="msum"
                            )
                            nc.vector.tensor_mul(msum, ssum4, mask)
                            red_in = msum
                        else:
                            # overlapped tail: per-row exp, junk rows never
                            # touched
                            nc.vector.memset(ssum4, 0.0)
                            nc.vector.memset(expw, 0.0)
                            for s in range(SC):
                                nc.scalar.activation(
                                    expw[32 * s : 32 * s + 1, :],
                                    scores[32 * s : 32 * s + 1, :],
                                    mybir.ActivationFunctionType.Exp,
                                    accum_out=ssum4[32 * s : 32 * s + 1, 0:1],
                                )
                            red_in = ssum4
                        tot = smallp.tile([P, 1], f32, name=f"tot_b{b}",
                                          tag="tot")
                        nc.gpsimd.partition_all_reduce(
                            tot, red_in, 128, bass_isa.ReduceOp.add
                        )
                        rec = smallp.tile([P, 1], f32, name=f"rec_b{b}",
                                          tag="rec")
                        nc.vector.reciprocal(rec, tot)
                        outw = outp.tile(
                            [P, ST], f32, name=f"outw_b{b}", tag="outw"
                        )
                        nc.vector.tensor_scalar_mul(outw, expw, rec)
                        nc.sync.dma_start(
                            out=out_ap[b].rearrange("(r s) -> r s", r=SC),
                            in_=outw[0 : 32 * (SC - 1) + 1 : 32, :],
                        )
                    return tail

                tails.append(make_tail())

            flush_and_tail(0)
            flush_and_tail(1, pop=True)
            flush_and_tail(0, pop=True)

    nc.compile()
    return nc


def _get_nc():
    global _COMPILED
    if _COMPILED is None:
        _COMPILED = _build()
    return _COMPILED


def _prep_in_maps(hidden, encoder_outputs, attn_W, attn_b, v_w):
    import ml_dtypes

    hidden = np.asarray(hidden, dtype=np.float32)
    encoder_outputs = np.asarray(encoder_outputs, dtype=np.float32)
    attn_W = np.asarray(attn_W, dtype=np.float32)
    attn_b = np.asarray(attn_b, dtype=np.float32)
    v_w = np.asarray(v_w, dtype=np.float32)

    c_full = hidden @ attn_W[:D] + attn_b            # [B, D]
    we = np.ascontiguousarray(attn_W[D:]).astype(ml_dtypes.bfloat16)
    v = np.ascontiguousarray(v_w.reshape(DC, P).T).astype(ml_dtypes.bfloat16)
    vb = np.ascontiguousarray(
        np.repeat(v[:, :, None], ST, axis=2).reshape(P, DC * ST)
    )

    in_maps = []
    for i in range(N_CORES):
        lo = i * BL
        enc_t = np.ascontiguousarray(
            encoder_outputs[:, lo : lo + BL, :].transpose(1, 2, 0)
        ).astype(ml_dtypes.bfloat16)                 # [BL, E, S]
        c_shard = c_full[lo : lo + BL]               # [BL, D]
        c = np.ascontiguousarray(
            c_shard.reshape(BL, DC, P).transpose(2, 0, 1).reshape(P, BL * DC)
        )                                            # [P, BL*DC]
        in_maps.append({"enc_t": enc_t, "we": we, "c": c, "v": v,
                        "vb": vb})
    return in_maps


def run_full(inputs: dict, trace: bool = False):
    """Run on 8 cores; returns (full_output [B,S], BassKernelResults)."""
    nc = _get_nc()
    in_maps = _prep_in_maps(**inputs)
    res = run_bass_kernel_spmd(
        nc, in_maps, list(range(N_CORES)), trace=trace
    )
    out = np.concatenate(
        [res.results[i]["out"] for i in range(N_CORES)], axis=0
    )
    return out, res


def kernel(**inputs) -> np.ndarray:
    out, _ = run_full(inputs)
    return out


# revision 23
# speedup vs baseline: 1.0657x; 1.0657x over previous
"""Bahdanau-style attention scores kernel for 8 TRN2 NeuronCores.

Reference math (B=64, S=2048, E=512, D=512):
    Wh = attn_W[:D]; We = attn_W[D:]
    h_proj = hidden @ Wh                                  # [B, D]
    e_proj[b,s,:] = enc[b,s,:] @ We                       # [B, S, D]
    energy = tanh(h_proj[:,None,:] + e_proj + attn_b)     # [B, S, D]
    scores = energy @ v_w                                 # [B, S]
    out = softmax(scores, axis=1)

Sharding: data-parallel over batch, 8 batches per core.
Host precomputes c = hidden @ Wh + attn_b (tiny: 33 MFLOP), converts
enc/We to bf16 (halves HBM traffic; absmax_rel ~5e-3, gate is 2e-2),
and transposes enc to [b, e, s] so the contraction axis e lands on SBUF
partitions.

Per-core kernel (b = 0..7 local batches), d-outer:
  for d: 16 matmuls (4e x 4s) accumulate e_projT[d_chunk, s] into
         2-bank PSUM pair tiles. d>0 runs e-outer/s-inner so consecutive
         matmuls share lhsT; d=0 runs s-outer/e-inner so the first
         matmuls only depend on the first enc s-block DMA.
  energyT = tanh(psum + c_b[d]) -> bf16, one ACT call per [128, 1024]
  score matvecs v_d.T @ energyT accumulate into PSUM rows 0/32/64/96
  (tile_position col strips), deferred one d so they never head-of-line
  block the main matmuls.
  softmax tail on the 4 score rows, deferred one batch for overlap.
A few dummy warm-up matmuls at t=0 lift the PE HAM clock gate toward
2.4 GHz before the first real matmul's DMA dependencies land; the
startup DMAs are split fine-grained (enc/We half-chunks) so the first
matmul starts ~2 us earlier.
"""

import numpy as np

import concourse.bass as bass  # noqa: F401  (engine namespaces via nc)
import concourse.mybir as mybir
import concourse.tile as tile
from concourse import bacc, bass_isa
from concourse.bass_utils import run_bass_kernel_spmd

N_CORES = 8
B, S, E, D = 64, 2048, 512, 512
BL = B // N_CORES          # local batches per core
P = 128                    # partition tile
EC = E // P                # e chunks (4)
DC = D // P                # d chunks (4)
ST = 512                   # s tile (free dim per matmul; one PSUM bank f32)
SC = S // ST               # s tiles (4)

DT = mybir.dt.bfloat16     # matmul input dtype (enc, We, v, energy)

_COMPILED = None  # nc cache within the process


def _build(warmup=5, enc_bufs=3, psp_bufs=3, en_bufs=6, mv="pe"):
    nc = bacc.Bacc(
        "TRN2", target_bir_lowering=False, debug=False, num_devices=N_CORES
    )
    f32 = mybir.dt.float32

    enc_ap = nc.dram_tensor("enc_t", [BL, E, S], DT, kind="ExternalInput").ap()
    we_ap = nc.dram_tensor("we", [E, D], DT, kind="ExternalInput").ap()
    c_ap = nc.dram_tensor("c", [P, BL * DC], f32, kind="ExternalInput").ap()
    v_ap = nc.dram_tensor("v", [P, DC], DT, kind="ExternalInput").ap()
    vb_ap = nc.dram_tensor("vb", [P, DC * ST], DT, kind="ExternalInput").ap()
    out_ap = nc.dram_tensor("out", [BL, S], f32, kind="ExternalOutput").ap()

    with tile.TileContext(nc) as tc:
        with (
            tc.tile_pool(name="singles", bufs=1) as singles,
            tc.tile_pool(name="encp", bufs=enc_bufs) as encp,
            tc.tile_pool(name="enp", bufs=en_bufs) as enp,
            tc.tile_pool(name="envp", bufs=4) as envp,
            tc.tile_pool(name="accp", bufs=8) as accp,
            tc.tile_pool(name="expp", bufs=2) as expp,
            tc.tile_pool(name="smallp", bufs=2) as smallp,
            tc.tile_pool(name="outp", bufs=2) as outp,
            tc.tile_pool(name="warmp", bufs=1) as warmp,
            tc.tile_pool(name="psp", bufs=psp_bufs, space="PSUM") as psp,
            tc.tile_pool(name="scp", bufs=2, space="PSUM") as scp,
        ):
            # ---- PE warm-up: run the HAM activity window hot before the
            # first real matmul's DMA deps arrive (dummy data, never read).
            # The zeros tile doubles as the lhsT of the per-batch score-bank
            # zeroing matmul. gpsimd does the memset (its preamble retires
            # earliest; DVE's is ~1 us later).
            wsb = warmp.tile([P, ST], DT)
            nc.gpsimd.memset(wsb, 0.0)
            wps = scp.tile([P, ST], f32, name="warm_ps", tag="sc")
            for _ in range(warmup):
                nc.tensor.matmul(wps, lhsT=wsb[:, 0:P], rhs=wsb,
                                 start=True, stop=True)
            # 0/1 mask of the four score rows (partitions 0/32/64/96)
            mask = warmp.tile([P, 1], f32, name="mask", tag="mask")
            nc.vector.memset(mask, 0.0)
            for s in range(SC):
                nc.vector.memset(mask[32 * s : 32 * s + 1, :], 1.0)

            we_sb = singles.tile([P, EC, D], DT)
            c_sb = singles.tile([P, BL * DC], f32)
            v_sb = singles.tile([P, DC], DT)
            if mv == "dve":
                # v broadcast along the free axis, for tensor_tensor muls
                vb_sb = singles.tile([P, DC, ST], DT)
                ones_sb = warmp.tile([P, 1], DT, name="ones", tag="ones")
                nc.gpsimd.memset(ones_sb, 1.0)

            pend = []   # deferred score matvecs, one d behind
            tails = []  # deferred per-batch softmax tails

            def flush_pend():
                for scores_t, s, d, en_t in pend:
                    if d == 0 and s == 0:
                        # zero the whole score bank (and set has_written)
                        # so the tail's exp can read all 128 rows in one go
                        nc.tensor.matmul(
                            scores_t, lhsT=wsb[:, 0:P], rhs=wsb,
                            start=True, stop=False,
                        )
                    nc.tensor.matmul(
                        scores_t[32 * s : 32 * s + 1, :],
                        lhsT=v_sb[:, d : d + 1],
                        rhs=en_t,
                        start=False,
                        stop=False,
                        tile_position=(0, 32 * s),
                    )
                    if d == DC - 1 and s == SC - 1:
                        # accumulate-zero closer: data no-op, marks every
                        # element's accumulation group closed
                        nc.tensor.matmul(
                            scores_t, lhsT=wsb[:, 0:P], rhs=wsb,
                            start=False, stop=True,
                        )
                pend.clear()

            def flush_pend_dve():
                # one partition-reducing ones-matvec per (batch, s-tile)
                for scores_t, s, acc_t, last_b in pend:
                    if last_b and s == 0:
                        nc.tensor.matmul(
                            scores_t, lhsT=wsb[:, 0:P], rhs=wsb,
                            start=True, stop=False,
                        )
                    nc.tensor.matmul(
                        scores_t[32 * s : 32 * s + 1, :],
                        lhsT=ones_sb[:, 0:1],
                        rhs=acc_t,
                        start=not last_b,
                        stop=not last_b,
                        tile_position=(0, 32 * s),
                    )
                    if last_b and s == SC - 1:
                        nc.tensor.matmul(
                            scores_t, lhsT=wsb[:, 0:P], rhs=wsb,
                            start=False, stop=True,
                        )
                pend.clear()

            def flush_and_tail(pop=True):
                flush_pend()
                if pop and tails:
                    tails.pop(0)()

            we_src = we_ap.rearrange("(e p) d -> p e d", p=P)

            for b in range(BL):
                enc_sb = encp.tile([P, EC, S], DT, name=f"enc_b{b}", tag="enc")
                enc_src = enc_ap[b].rearrange("(e p) s -> p e s", p=P)
                if b == 0:
                    # fine-grained startup: alternate enc-s0 / We half
                    # chunks so the first matmul's deps land after 256 KB,
                    # then c/v (small), then the remaining s-blocks
                    for e2 in range(2):
                        nc.sync.dma_start(
                            out=enc_sb[:, 2 * e2 : 2 * e2 + 2, 0:ST],
                            in_=enc_src[:, 2 * e2 : 2 * e2 + 2, 0:ST],
                        )
                        nc.sync.dma_start(
                            out=we_sb[:, 2 * e2 : 2 * e2 + 2, :],
                            in_=we_src[:, 2 * e2 : 2 * e2 + 2, :],
                        )
                    nc.sync.dma_start(out=c_sb, in_=c_ap)
                    nc.sync.dma_start(out=v_sb, in_=v_ap)
                    if mv == "dve":
                        nc.sync.dma_start(
                            out=vb_sb,
                            in_=vb_ap.rearrange("p (d t) -> p d t", d=DC),
                        )
                    for s in range(1, SC):
                        nc.sync.dma_start(
                            out=enc_sb[:, :, s * ST : (s + 1) * ST],
                            in_=enc_src[:, :, s * ST : (s + 1) * ST],
                        )
                else:
                    for h in range(2):
                        sl = slice(h * 2 * ST, (h + 1) * 2 * ST)
                        nc.sync.dma_start(
                            out=enc_sb[:, :, sl], in_=enc_src[:, :, sl]
                        )

                scores = scp.tile([P, ST], f32, name=f"scores_b{b}", tag="sc")
                if mv == "dve":
                    accs = [
                        accp.tile([P, ST], DT, name=f"acc_b{b}s{s}",
                                  tag="acc")
                        for s in range(SC)
                    ]

                for d in range(DC):
                    ps_pair = [
                        psp.tile([P, 2, ST], f32, name=f"ps_b{b}d{d}h{h}",
                                 tag="ps")
                        for h in range(2)
                    ]

                    def mm(e, s):
                        nc.tensor.matmul(
                            ps_pair[s // 2][:, s % 2, :],
                            lhsT=we_sb[:, e, d * P : (d + 1) * P],
                            rhs=enc_sb[:, e, s * ST : (s + 1) * ST],
                            start=(e == 0),
                            stop=(e == EC - 1),
                        )

                    if b == 0 and d == 0:
                        # s-outer: first matmuls only need the first enc
                        # s-block
                        order = [(e, s) for s in range(SC)
                                 for e in range(EC)]
                    else:
                        # e-outer: 4 consecutive matmuls share lhsT
                        order = [(e, s) for e in range(EC)
                                 for s in range(SC)]
                    for i, (e, s) in enumerate(order):
                        mm(e, s)
                        if i == 13:
                            # flush the previous d's score matvecs here:
                            # they fill the slot where the next matmuls
                            # would stall on psum rotation (h0's tanh is
                            # still draining)
                            flush_and_tail(d == 1)
                    for h in range(2):
                        en = enp.tile([P, 2, ST], DT, name=f"en_b{b}d{d}h{h}",
                                      tag="en")
                        nc.scalar.activation(
                            en,
                            ps_pair[h],
                            mybir.ActivationFunctionType.Tanh,
                            bias=c_sb[:, b * DC + d : b * DC + d + 1],
                        )
                        for g in range(2):
                            s = 2 * h + g
                            pend.append((scores, s, d, en[:, g, :]))

                def make_tail(b=b, scores=scores):
                    def tail():
                        ssum4 = smallp.tile(
                            [P, 1], f32, name=f"ssum4_b{b}", tag="ssum4"
                        )
                        expw = expp.tile(
                            [P, ST], f32, name=f"expw_b{b}", tag="expw"
                        )
                        if True:
                            # exposed tail: all 128 score rows are defined
                            # (zero opener/closer), so exp+accumulate runs
                            # as ONE call; junk-row partials (exp(0)*512)
                            # are masked out before the partition reduce
                            nc.scalar.activation(
                                expw,
                                scores,
                                mybir.ActivationFunctionType.Exp,
                                accum_out=ssum4[:, 0:1],
                            )
                            msum = smallp.tile(
                                [P, 1], f32, name=f"msum_b{b}", tag="msum"
                            )
                            nc.vector.tensor_mul(msum, ssum4, mask)
                            red_in = msum
                        else:
                            # overlapped tail: per-row exp, junk rows never
                            # touched
                            nc.vector.memset(ssum4, 0.0)
                            nc.vector.memset(expw, 0.0)
                            for s in range(SC):
                                nc.scalar.activation(
                                    expw[32 * s : 32 * s + 1, :],
                                    scores[32 * s : 32 * s + 1, :],
                                    mybir.ActivationFunctionType.Exp,
                                    accum_out=ssum4[32 * s : 32 * s + 1, 0:1],
                                )
                            red_in = ssum4
                        tot = smallp.tile([P, 1], f32, name=f"tot_b{b}",
                                          tag="tot")
                        nc.gpsimd.partition_all_reduce(
                            tot, red_in, 128, bass_isa.ReduceOp.add
                        )
                        rec = smallp.tile([P, 1], f32, name=f"rec_b{b}",
                                          tag="rec")
                        nc.vector.reciprocal(rec, tot)
                        outw = outp.tile(
                            [P, ST], f32, name=f"outw_b{b}", tag="outw"
                        )
                        nc.vector.tensor_scalar_mul(outw, expw, rec)
                        nc.sync.dma_start(
                            out=out_ap[b].rearrange("(r s) -> r s", r=SC),
                            in_=outw[0 : 32 * (SC - 1) + 1 : 32, :],
                        )
                    return tail

                tails.append(make_tail())

            flush_and_tail()
            flush_and_tail()

    nc.compile()
    return nc


def _get_nc():
    global _COMPILED
    if _COMPILED is None:
        _COMPILED = _build()
    return _COMPILED


def _prep_in_maps(hidden, encoder_outputs, attn_W, attn_b, v_w):
    import ml_dtypes

    hidden = np.asarray(hidden, dtype=np.float32)
    encoder_outputs = np.asarray(encoder_outputs, dtype=np.float32)
    attn_W = np.asarray(attn_W, dtype=np.float32)
    attn_b = np.asarray(attn_b, dtype=np.float32)
    v_w = np.asarray(v_w, dtype=np.float32)

    c_full = hidden @ attn_W[:D] + attn_b            # [B, D]
    we = np.ascontiguousarray(attn_W[D:]).astype(ml_dtypes.bfloat16)
    v = np.ascontiguousarray(v_w.reshape(DC, P).T).astype(ml_dtypes.bfloat16)
    vb = np.ascontiguousarray(
        np.repeat(v[:, :, None], ST, axis=2).reshape(P, DC * ST)
    )

    in_maps = []
    for i in range(N_CORES):
        lo = i * BL
        enc_t = np.ascontiguousarray(
            encoder_outputs[:, lo : lo + BL, :].transpose(1, 2, 0)
        ).astype(ml_dtypes.bfloat16)                 # [BL, E, S]
        c_shard = c_full[lo : lo + BL]               # [BL, D]
        c = np.ascontiguousarray(
            c_shard.reshape(BL, DC, P).transpose(2, 0, 1).reshape(P, BL * DC)
        )                                            # [P, BL*DC]
        in_maps.append({"enc_t": enc_t, "we": we, "c": c, "v": v,
                        "vb": vb})
    return in_maps


def run_full(inputs: dict, trace: bool = False):
    """Run on 8 cores; returns (full_output [B,S], BassKernelResults)."""
    nc = _get_nc()
    in_maps = _prep_in_maps(**inputs)
    res = run_bass_kernel_spmd(
        nc, in_maps, list(range(N_CORES)), trace=trace
    )
    out = np.concatenate(
        [res.results[i]["out"] for i in range(N_CORES)], axis=0
    )
    return out, res


def kernel(**inputs) -> np.ndarray:
    out, _ = run_full(inputs)
    return out
